# revision 3
# baseline (speedup 1.0000x reference)
"""ConvPooler kernel for Trainium2 (8 NeuronCores, data-parallel over batch).

Computes: vals[b,s] = h[b,s,:] @ w + b ; out[b, gene_pos[b,s]] = vals[b,s]
with out zero-initialized, shape [32, 60000]; pad id 60000 is dropped.

Sharding: batch dim (32 rows) split 4 rows per core; w/b replicated.
Per-core device program:
  - DVE: fused multiply+reduce (tensor_tensor_reduce) over [128,512] slices
    of 1MB h tiles -> vals [128, 64] (token (4p+s) of chunk t -> col 4t+s).
  - GPSIMD indirect DMA: per batch row, scatter 2048 scalars into a
    zero-filled [60032] DRAM row buffer using host-prepared int32 indices.
"""

import sys

sys.path.insert(0, "/opt/trn_rl_repo")

import numpy as np

import concourse.bacc as bacc
import concourse.bass as bass
import concourse.mybir as mybir
import concourse.tile as tile
from concourse.bass_utils import run_bass_kernel_spmd

B, S, D = 32, 2048, 512
FULL = 60000
ROWP = 60032  # padded row width: 469 * 128, >= FULL + 1 (pad id lands in slack)
NCORES = 8
RPC = B // NCORES  # batch rows per core
TOK = RPC * S  # tokens per core
CHUNK = 512  # tokens per h DMA tile: [128, 2048] f32 = 1 MiB
NT = TOK // CHUNK  # h tiles per core
COLS = TOK // 128  # vals columns per core
CPR = S // 128  # vals columns per batch row

_cached = None


def _build_program():
    nc = bacc.Bacc(
        "TRN2",
        target_bir_lowering=False,
        debug=False,
        num_devices=NCORES,
    )
    hs = nc.dram_tensor("hs", [TOK, D], mybir.dt.float32, kind="ExternalInput")
    wb = nc.dram_tensor("wb", [128, D], mybir.dt.float32, kind="ExternalInput")
    bb = nc.dram_tensor("bb", [128, 1], mybir.dt.float32, kind="ExternalInput")
    idx = nc.dram_tensor("idx", [128, COLS], mybir.dt.int32, kind="ExternalInput")
    outs = [
        nc.dram_tensor(f"out{r}", [ROWP, 1], mybir.dt.float32, kind="ExternalOutput")
        for r in range(RPC)
    ]

    with tile.TileContext(nc) as tc:
        with (
            tc.tile_pool(name="const", bufs=1) as cpool,
            tc.tile_pool(name="hload", bufs=4) as hpool,
            tc.tile_pool(name="prod", bufs=2) as ppool,
            tc.tile_pool(name="vals", bufs=1) as vpool,
        ):
            w_t = cpool.tile([128, D], mybir.dt.float32)
            nc.sync.dma_start(out=w_t[:], in_=wb.ap())
            b_t = cpool.tile([128, 1], mybir.dt.float32)
            nc.sync.dma_start(out=b_t[:], in_=bb.ap())
            i_t = cpool.tile([128, COLS], mybir.dt.int32)
            nc.sync.dma_start(out=i_t[:], in_=idx.ap())

            z_t = cpool.tile([128, ROWP // 128], mybir.dt.float32)
            nc.vector.memset(z_t[:], 0.0)
            for r in range(RPC):
                nc.sync.dma_start(out=outs[r].ap(), in_=z_t[:])

            vals = vpool.tile([128, COLS], mybir.dt.float32)
            for t in range(NT):
                h_t = hpool.tile([128, CHUNK * D // 128], mybir.dt.float32)
                nc.sync.dma_start(out=h_t[:], in_=hs.ap()[t * CHUNK : (t + 1) * CHUNK, :])
                for s in range(CHUNK * D // 128 // D):
                    c = (CHUNK // 128) * t + s
                    prod = ppool.tile([128, D], mybir.dt.float32)
                    nc.vector.tensor_tensor(
                        out=prod[:],
                        in0=h_t[:, s * D : (s + 1) * D],
                        in1=w_t[:],
                        op=mybir.AluOpType.mult,
                    )
                    dead = ppool.tile([128, D], mybir.dt.float32, tag="dead")
                    nc.scalar.activation(
                        out=dead[:],
                        in_=prod[:],
                        func=mybir.ActivationFunctionType.Copy,
                        accum_out=vals[:, c : c + 1],
                    )
                # after the last chunk of a batch row, add bias and scatter the
                # row: HW indirect DMA pairs ONE index per partition row, so
                # each scatter moves a [128, 1] column (NKI router_topk pattern)
                if (t + 1) % (NT // RPC) == 0:
                    r = t // (NT // RPC)
                    rs = slice(r * CPR, (r + 1) * CPR)
                    nc.vector.tensor_scalar(
                        out=vals[:, rs],
                        in0=vals[:, rs],
                        scalar1=b_t[:],
                        scalar2=None,
                        op0=mybir.AluOpType.add,
                    )
                    flat = bass.AP(outs[r], 0, [[1, ROWP], [1, 1]])
                    for c in range(r * CPR, (r + 1) * CPR):
                        nc.gpsimd.indirect_dma_start(
                            out=flat,
                            out_offset=bass.IndirectOffsetOnAxis(
                                ap=i_t[:, c : c + 1], axis=0
                            ),
                            in_=vals[:, c : c + 1],
                            in_offset=None,
                        )

    nc.compile()
    return nc


def _get_program():
    global _cached
    if _cached is None:
        _cached = _build_program()
    return _cached


def _make_in_maps(h, gene_pos, w, b):
    h = np.ascontiguousarray(np.asarray(h, dtype=np.float32))
    gp = np.asarray(gene_pos).astype(np.int32)
    w = np.asarray(w, dtype=np.float32)
    b = np.asarray(b, dtype=np.float32)

    wb = np.ascontiguousarray(np.broadcast_to(w, (128, D)))
    bb = np.full((128, 1), b[0], dtype=np.float32)

    in_maps = []
    for c in range(NCORES):
        hsub = h[c * RPC : (c + 1) * RPC].reshape(TOK, D)
        gsub = gp[c * RPC : (c + 1) * RPC]  # [RPC, S]
        # tile layout: idx[p, 16r + 4tt + s] = gene_pos[r, 512*tt + 4p + s]
        idxc = np.ascontiguousarray(
            gsub.reshape(RPC, S // CHUNK, 128, CHUNK // 128)
            .transpose(2, 0, 1, 3)
            .reshape(128, COLS)
        )
        in_maps.append({"hs": hsub, "wb": wb, "bb": bb, "idx": idxc})
    return in_maps


def kernel(h, gene_pos, w, b):
    nc = _get_program()
    in_maps = _make_in_maps(h, gene_pos, w, b)
    res = run_bass_kernel_spmd(nc, in_maps, core_ids=list(range(NCORES)))
    full = np.empty((B, FULL), dtype=np.float32)
    for c in range(NCORES):
        for r in range(RPC):
            full[c * RPC + r] = res.results[c][f"out{r}"].reshape(ROWP)[:FULL]
    return full
